# revision 8
# baseline (speedup 1.0000x reference)
"""Trainium2 Bass kernel for nn_MinConv2dGRU.

Reference model: two layers of [Conv2d(3x3, pad 1) -> minGRU scan over S].
  x: (B=4, S=64, C=32, H=32, W=32) f32
  layer L: conv -> (gate, hidden) each 64ch; z = sigmoid(gate);
           h_t = (1-z_t) h_{t-1} + z_t * g(hidden_t), h_0 = 0.5
           g(x) = x+0.5 if x>=0 else sigmoid(x)  ==  max(sigmoid(x), x+0.5)
  outputs: (out2, h1, h2) with h1 = out1[:, -1:], h2 = out2[:, -1:]

Sharding: 8 cores = B(4) x H-halves(2). Each core computes 16 output rows
(+1 halo row of layer-1 state) from 18 input rows; the bottom half runs on
vertically flipped data with dy-flipped weights so a single SPMD program
serves all 8 cores. No cross-core communication.

Per-core pipeline (streaming over the 64 frames, layer-2 skewed by one
frame so the PE never waits on the scan chain):
  conv1: fp32r matmuls, K=97 = (dy in {-1,0,1}) x 32ch + ones-row (bias
         folded into the weights). Moving operand is host-prepped
         "dy-im2col": (97, 17 rows, 34 cols) with zero padding baked in.
         PSUM (128,17,32) spans 2 banks; per dx one N=512 matmul (rows
         0..15) + one N=32 matmul (row 16).
  scan1: ACT: sig = sigmoid(P1) (128p: z | s), t = P1[64:128] + 0.5.
         DVE (scalar_tensor_tensor, 2x fp32-SBUF mode):
           g = max(s, t); d = g - h_prev; m = z*d; h = h_prev + m
         h written into T2a[64:128] (dy=0 group, h1 row r at pos r);
         replica into T2a[0:64] (dy=-1 group, pos r+1) on gpsimd.
         The dy=+1 tap reads T2a[64:128] at pos+1 - no third copy.
  conv2: PSUM (128,16,32) = 1 bank; 3 dx x (K=128 dy{-1,0} + K=64 dy=+1),
         all N=512.
  scan2: ACT: sig2 = sigmoid(P2 + b2); DVE: g2 = (P2[64:128]+b2t) max s2,
         d2, m2; gpsimd: o2 = h2_prev + m2; DMA out. o2 doubles as h2
         state for the next frame.
"""

import os
import sys

import numpy as np

sys.path.insert(0, "/opt/trn_rl_repo")

B, S, C, H, W = 4, 64, 32, 32, 32
HC = 64  # hidden channels per layer
NCORES = 8
G_ZERO = 0.5

# set by test.py to get profiling
TRACE = bool(int(os.environ.get("KERNEL_TRACE", "0")))
LAST_RESULTS = None

_NC_CACHE = None


def _build_nc():
    import concourse.bacc as bacc
    import concourse.tile as tile
    from concourse import mybir

    f32 = mybir.dt.float32
    f32r = mybir.dt.float32r

    nc = bacc.Bacc(
        "TRN2", target_bir_lowering=False, debug=False, num_devices=NCORES
    )

    # ---- DRAM parameters -------------------------------------------------
    xp = nc.dram_tensor("xp", [S, 97, 17, 34], f32r, kind="ExternalInput").ap()
    w1s = nc.dram_tensor("w1s", [3, 97, 128], f32r, kind="ExternalInput").ap()
    w2a = nc.dram_tensor("w2a", [3, 128, 128], f32r, kind="ExternalInput").ap()
    w2b = nc.dram_tensor("w2b", [3, 64, 128], f32r, kind="ExternalInput").ap()
    b2v = nc.dram_tensor("b2v", [128, 1], f32, kind="ExternalInput").ap()
    b2t = nc.dram_tensor("b2t", [64, 1], f32, kind="ExternalInput").ap()
    out2o = nc.dram_tensor("out2o", [S, 64, 16, 32], f32, kind="ExternalOutput").ap()
    h1o = nc.dram_tensor("h1o", [64, 17, 32], f32r, kind="ExternalOutput").ap()

    with tile.TileContext(nc) as tc:
        _emit(tc, nc, mybir, xp, w1s, w2a, w2b, b2v, b2t, out2o, h1o)

    nc.compile()
    return nc


def _emit(tc, nc, mybir, xp, w1s, w2a, w2b, b2v, b2t, out2o, h1o):
    from contextlib import ExitStack

    f32 = mybir.dt.float32
    f32r = mybir.dt.float32r
    SIG = mybir.ActivationFunctionType.Sigmoid
    COPY = mybir.ActivationFunctionType.Copy
    BYP = mybir.AluOpType.bypass
    ADD = mybir.AluOpType.add
    SUB = mybir.AluOpType.subtract
    MUL = mybir.AluOpType.mult
    MAX = mybir.AluOpType.max

    ctx = ExitStack()
    with ctx:
        singles = ctx.enter_context(tc.tile_pool(name="singles", bufs=1))
        xpool = ctx.enter_context(tc.tile_pool(name="xpool", bufs=3))
        p1_pool = ctx.enter_context(tc.tile_pool(name="p1", bufs=2, space="PSUM"))
        p2_pool = ctx.enter_context(tc.tile_pool(name="p2", bufs=2, space="PSUM"))
        spool = ctx.enter_context(tc.tile_pool(name="spool", bufs=2))
        tpool = ctx.enter_context(tc.tile_pool(name="tpool", bufs=2))
        gpool = ctx.enter_context(tc.tile_pool(name="gpool", bufs=2))
        dpool = ctx.enter_context(tc.tile_pool(name="dpool", bufs=2))
        mpool = ctx.enter_context(tc.tile_pool(name="mpool", bufs=2))
        t2pool = ctx.enter_context(tc.tile_pool(name="t2pool", bufs=3))
        opool = ctx.enter_context(tc.tile_pool(name="opool", bufs=3))

        # ---- resident constants -----------------------------------------
        w1_sb = singles.tile([97, 3, 128], f32r)
        nc.sync.dma_start(out=w1_sb, in_=w1s.rearrange("d k m -> k d m"))
        w2a_sb = singles.tile([128, 3, 128], f32r)
        nc.sync.dma_start(out=w2a_sb, in_=w2a.rearrange("d k m -> k d m"))
        # lives in partitions 64:128 so lhsT/rhs base partitions match for
        # the dy=+1 tap (which reads t2a[64:128] shifted)
        w2b_sb = singles.tile([128, 3, 128], f32r)
        nc.sync.dma_start(out=w2b_sb[64:128], in_=w2b.rearrange("d k m -> k d m"))
        b2v_sb = singles.tile([128, 1], f32)
        nc.sync.dma_start(out=b2v_sb, in_=b2v)
        b2t_sb = singles.tile([128, 1], f32)
        nc.sync.dma_start(out=b2t_sb[64:128], in_=b2t)
        # initial states: scan home is partitions 64:128; the dy=-1 replica
        # chain (rows 0..14) lives at partitions 0:64
        hinit = singles.tile([128, 17, 32], f32)
        nc.vector.memset(hinit[64:128], G_ZERO)
        hinit0 = singles.tile([64, 15, 32], f32)
        nc.vector.memset(hinit0, G_ZERO)
        h2init = singles.tile([128, 16, 32], f32)
        nc.vector.memset(h2init[64:128], G_ZERO)

        h1_prev = hinit[64:128, :, :]    # layer-1 state (parts 64:128)
        h0_prev = hinit0[:, :, :]        # dy=-1 replica state (parts 0:64)
        h2_prev = h2init[64:128, :, :]   # layer-2 state (parts 64:128)
        carry = None                     # one-frame-skew t2a tile

        def emit_conv2(t2a):
            p2 = p2_pool.tile([128, 16, 32], f32)
            first = True
            for dx in range(3):
                nc.tensor.matmul(
                    p2, w2a_sb[:, dx, :], t2a[:, 0:16, dx:dx + 32],
                    start=first, stop=False)
                first = False
                nc.tensor.matmul(
                    p2, w2b_sb[64:128, dx, :], t2a[64:128, 1:17, dx:dx + 32],
                    start=False, stop=(dx == 2))
            return p2

        def emit_scan2(p2, t):
            nonlocal h2_prev
            sig2 = spool.tile([128, 16, 32], f32, tag="sig2")
            nc.scalar.activation(sig2, p2, SIG, bias=b2v_sb)
            z2t = spool.tile([128, 16, 32], f32, tag="z2t")
            nc.sync.dma_start(out=z2t[64:128], in_=sig2[0:64])
            g2 = gpool.tile([128, 16, 32], f32, tag="g2")
            nc.vector.scalar_tensor_tensor(
                g2[64:128], p2[64:128], b2t_sb[64:128], sig2[64:128], ADD, MAX)
            d2 = dpool.tile([128, 16, 32], f32, tag="d2")
            nc.vector.scalar_tensor_tensor(
                d2[64:128], g2[64:128], 0.0, h2_prev, BYP, SUB)
            m2 = mpool.tile([128, 16, 32], f32, tag="m2")
            nc.vector.scalar_tensor_tensor(
                m2[64:128], z2t[64:128], 0.0, d2[64:128], BYP, MUL)
            o2 = opool.tile([128, 16, 32], f32)
            nc.gpsimd.tensor_add(o2[64:128], h2_prev, m2[64:128])
            nc.sync.dma_start(out=out2o[t], in_=o2[64:128])
            h2_prev = o2[64:128, :, :]

        for t in range(S):
            # ---- conv1 (frame t) ----------------------------------------
            xt = xpool.tile([97, 17, 34], f32r)
            nc.sync.dma_start(out=xt, in_=xp[t])
            p1 = p1_pool.tile([128, 17, 32], f32)
            for dx in range(3):
                nc.tensor.matmul(
                    p1[:, 0:16, :], w1_sb[:, dx, :], xt[:, 0:16, dx:dx + 32],
                    start=(dx == 0), stop=(dx == 2))
            for dx in range(3):
                nc.tensor.matmul(
                    p1[:, 16:17, :], w1_sb[:, dx, :], xt[:, 16:17, dx:dx + 32],
                    start=(dx == 0), stop=(dx == 2))

            # ---- conv2 (frame t-1, skewed so PE never stalls) -----------
            if carry is not None:
                p2 = emit_conv2(carry)

            # ---- scan1 (frame t), homed at partitions 64:128 ------------
            sig = spool.tile([128, 17, 32], f32)
            nc.scalar.activation(sig, p1, SIG)
            # z lives at 0:64 (gate channels); hop it to the scan home
            z64 = spool.tile([128, 17, 32], f32, tag="z64")
            nc.sync.dma_start(out=z64[64:128], in_=sig[0:64])
            tt = tpool.tile([128, 17, 32], f32)
            nc.scalar.activation(tt[64:128], p1[64:128], COPY, bias=0.5)

            g = gpool.tile([128, 17, 32], f32)
            nc.vector.scalar_tensor_tensor(
                g[64:128], sig[64:128], 0.0, tt[64:128], BYP, MAX)
            d = dpool.tile([128, 17, 32], f32)
            nc.vector.scalar_tensor_tensor(
                d[64:128], g[64:128], 0.0, h1_prev, BYP, SUB)
            m = mpool.tile([128, 17, 32], f32)
            nc.vector.scalar_tensor_tensor(
                m[64:128], z64[64:128], 0.0, d[64:128], BYP, MUL)

            # conv2 moving tile: partitions 0:64 = dy=-1 group (h1 row r at
            # pos r+1), 64:128 = dy=0 group (h1 row r at pos r).
            t2a = t2pool.tile([128, 18, 34], f32r)
            # (bitcast: walrus rejects memset on f32r)
            nc.vector.memset(t2a[:, :, 0:34:33].bitcast(f32), 0.0)  # pad cols
            nc.vector.memset(t2a[0:64, 0, 1:33].bitcast(f32), 0.0)  # dy=-1 pos 0
            nc.vector.scalar_tensor_tensor(
                t2a[64:128, 0:17, 1:33], h1_prev, 0.0, m[64:128], BYP, ADD)
            # dy=-1 replica chain at parts 0:64 (conv2 reads rows 0..14 at
            # pos 1..15): hop m down, then local add
            m0 = mpool.tile([64, 15, 32], f32, tag="m0")
            nc.sync.dma_start(out=m0, in_=m[64:128, 0:15, :])
            nc.gpsimd.tensor_add(t2a[0:64, 1:16, 1:33], h0_prev, m0)
            h1_prev = t2a[64:128, 0:17, 1:33]
            h0_prev = t2a[0:64, 1:16, 1:33]

            # ---- scan2 (frame t-1) --------------------------------------
            if carry is not None:
                emit_scan2(p2, t - 1)
            carry = t2a

        # tail: layer 2 for the last frame
        p2 = emit_conv2(carry)
        emit_scan2(p2, S - 1)

        nc.sync.dma_start(out=h1o, in_=carry[64:128, 0:17, 1:33])


def _prep_core_inputs(x, w1, b1, w2, b2, core):
    """Build the per-core input map (numpy, top-core semantics)."""
    b = core % 4
    half = core // 4
    if half == 0:
        xl = x[b][:, :, 0:18, :]                      # (S, C, 18, W)
        w1e, w2e = w1, w2
    else:
        xl = x[b][:, :, :13:-1, :]                    # rows 31..14 flipped
        w1e = w1[:, :, ::-1, :]
        w2e = w2[:, :, ::-1, :]

    # dy-im2col with zero pads + ones-row (bias):
    #   xp[t, dyi*32+c, y, 1+xx] = xl[t, c, y-1+dyi, xx];  xp[t, 96] = 1
    xpad = np.zeros((S, C, 19, W + 2), np.float32)
    xpad[:, :, 1:19, 1:33] = xl
    xpc = np.empty((S, 97, 17, 34), np.float32)
    for dyi in range(3):
        xpc[:, dyi * 32:(dyi + 1) * 32] = xpad[:, :, dyi:dyi + 17, :]
    xpc[:, 96] = 1.0

    # lhsT weights: [dx, (dyi, c), o]; bias row only in dx=1
    w1t = np.zeros((3, 97, 128), np.float32)
    w1t[:, 0:96] = w1e.transpose(3, 2, 1, 0).reshape(3, 96, 128)
    w1t[1, 96] = b1

    w2t = w2e.transpose(3, 2, 1, 0)                   # (dx, dy, c, o)
    w2at = np.ascontiguousarray(
        w2t[:, 0:2].reshape(3, 128, 128).astype(np.float32))
    w2bt = np.ascontiguousarray(w2t[:, 2].astype(np.float32))

    return {
        "xp": np.ascontiguousarray(xpc),
        "w1s": w1t,
        "w2a": w2at,
        "w2b": w2bt,
        "b2v": b2.reshape(128, 1).astype(np.float32),
        "b2t": (b2[64:128] + 0.5).reshape(64, 1).astype(np.float32),
    }


def kernel(x, w1, b1, w2, b2):
    global _NC_CACHE, LAST_RESULTS
    from concourse.bass_utils import run_bass_kernel_spmd

    x = np.asarray(x, dtype=np.float32)
    w1 = np.asarray(w1, dtype=np.float32)
    b1 = np.asarray(b1, dtype=np.float32)
    w2 = np.asarray(w2, dtype=np.float32)
    b2 = np.asarray(b2, dtype=np.float32)

    if _NC_CACHE is None:
        _NC_CACHE = _build_nc()
    nc = _NC_CACHE

    in_maps = [_prep_core_inputs(x, w1, b1, w2, b2, i) for i in range(NCORES)]
    res = run_bass_kernel_spmd(nc, in_maps, list(range(NCORES)), trace=TRACE)
    LAST_RESULTS = res

    out2 = np.empty((B, S, HC, H, W), np.float32)
    h1f = np.empty((B, 1, HC, H, W), np.float32)
    for core in range(NCORES):
        b = core % 4
        half = core // 4
        o = np.asarray(res.results[core]["out2o"], np.float32)
        h1s = np.asarray(res.results[core]["h1o"], np.float32).reshape(64, 17, 32)
        if half == 0:
            out2[b, :, :, 0:16] = o
            h1f[b, 0, :, 0:16] = h1s[:, 0:16]
        else:
            out2[b, :, :, 16:32] = o[:, :, ::-1, :]
            h1f[b, 0, :, 16:32] = h1s[:, 15::-1]
    h2 = out2[:, S - 1:S].copy()
    return out2, h1f, h2


# revision 23
# speedup vs baseline: 1.6695x; 1.6695x over previous
"""Trainium2 Bass kernel for nn_MinConv2dGRU.

Reference model: two layers of [Conv2d(3x3, pad 1) -> minGRU scan over S].
  x: (B=4, S=64, C=32, H=32, W=32) f32
  layer L: conv -> (gate, hidden) each 64ch; z = sigmoid(gate);
           h_t = (1-z_t) h_{t-1} + z_t * g(hidden_t), h_0 = 0.5
           g(x) = x+0.5 if x>=0 else sigmoid(x)  ==  max(sigmoid(x), x+0.5)
  outputs: (out2, h1, h2) with h1 = out1[:, -1:], h2 = out2[:, -1:]

Sharding: 8 cores = B(4) x H-halves(2). Each core computes 16 output rows
(+1 halo row of layer-1 state) from 18 input rows; the bottom half runs on
vertically flipped data with dy-flipped weights so a single SPMD program
serves all 8 cores. No cross-core communication.

Per-core pipeline (streaming over the 64 frames, layer-2 skewed by one
frame so the PE never stalls on the scan chain):
  conv1: bf16 matmuls, K=97 = (dy in {-1,0,1}) x 32ch + ones-row (bias
         folded into the weights). Moving operand is host-prepped
         "dy-im2col": (97, 17 rows, 34 cols) with zero padding baked in.
         PSUM (128,17,32) spans 2 banks; per dx one N=512 matmul (rows
         0..15) + one N=32 matmul (row 16).
  scan1 (homed at partitions 64:128; engines require in/out partition
         alignment, so z makes one DMA hop from 0:64 down to the home):
         ACT: sig = sigmoid(P1) (128p: z | s), t = P1[64:128] + 0.5.
         DVE: g = max(s, t); d = g - hst_prev; m = z64*d;
              hst = hst_prev + m   (contiguous state tile)
         gpsimd copies hst into the conv2 moving tile t2a[64:128]
         (h1 row r at pos r+1; pos 0 zero).
  conv2: 9 taps, all K=64 reading t2a[64:128] at shifted (pos, col)
         offsets; PSUM (128,16,32) = 1 bank, all N=512.
  scan2: ACT: sig2 = sigmoid(P2 + b2); DVE: g2 = (P2[64:128]+b2t) max s2,
         d2, m2; gpsimd: o2 = h2_prev + m2 into a 4-frame block tile
         (one out-DMA per 4 frames). o2 doubles as the h2 state.
"""

import os
import sys

import numpy as np

sys.path.insert(0, "/opt/trn_rl_repo")

B, S, C, H, W = 4, 64, 32, 32, 32
HC = 64  # hidden channels per layer
NCORES = 8
G_ZERO = 0.5

# set by test.py to get profiling
TRACE = bool(int(os.environ.get("KERNEL_TRACE", "0")))
LAST_RESULTS = None

_NC_CACHE = None


def _np_bf16():
    import ml_dtypes
    return ml_dtypes.bfloat16


def _build_nc():
    import concourse.bacc as bacc
    import concourse.tile as tile
    from concourse import mybir

    f32 = mybir.dt.float32
    bf16 = mybir.dt.bfloat16

    nc = bacc.Bacc(
        "TRN2", target_bir_lowering=False, debug=False, num_devices=NCORES
    )

    # ---- DRAM parameters -------------------------------------------------
    xp = nc.dram_tensor("xp", [S, 97, 17, 34], bf16, kind="ExternalInput").ap()
    w1s = nc.dram_tensor("w1s", [3, 97, 128], bf16, kind="ExternalInput").ap()
    w2a = nc.dram_tensor("w2a", [3, 128, 128], bf16, kind="ExternalInput").ap()
    w2b = nc.dram_tensor("w2b", [3, 64, 128], bf16, kind="ExternalInput").ap()
    b2v = nc.dram_tensor("b2v", [128, 1], f32, kind="ExternalInput").ap()
    b2t = nc.dram_tensor("b2t", [64, 1], f32, kind="ExternalInput").ap()
    out2o = nc.dram_tensor("out2o", [S, 64, 16, 32], bf16, kind="ExternalOutput").ap()
    h1o = nc.dram_tensor("h1o", [64, 17, 32], bf16, kind="ExternalOutput").ap()

    with tile.TileContext(nc) as tc:
        _emit(tc, nc, mybir, xp, w1s, w2a, w2b, b2v, b2t, out2o, h1o)

    nc.compile()
    return nc


def _emit(tc, nc, mybir, xp, w1s, w2a, w2b, b2v, b2t, out2o, h1o):
    from contextlib import ExitStack

    f32 = mybir.dt.float32
    bf16 = mybir.dt.bfloat16
    SIG = mybir.ActivationFunctionType.Sigmoid
    COPY = mybir.ActivationFunctionType.Copy
    ADD = mybir.AluOpType.add
    MAX = mybir.AluOpType.max

    OB = 8   # out2 frames per DMA

    ctx = ExitStack()
    with ctx:
        singles = ctx.enter_context(tc.tile_pool(name="singles", bufs=1))
        xpool = ctx.enter_context(tc.tile_pool(name="xpool", bufs=3))
        p1_pool = ctx.enter_context(tc.tile_pool(name="p1", bufs=2, space="PSUM"))
        p2_pool = ctx.enter_context(tc.tile_pool(name="p2", bufs=3, space="PSUM"))
        spool = ctx.enter_context(tc.tile_pool(name="spool", bufs=4))
        tpool = ctx.enter_context(tc.tile_pool(name="tpool", bufs=3))
        gpool = ctx.enter_context(tc.tile_pool(name="gpool", bufs=4))
        dpool = ctx.enter_context(tc.tile_pool(name="dpool", bufs=4))
        mpool = ctx.enter_context(tc.tile_pool(name="mpool", bufs=4))
        hpool = ctx.enter_context(tc.tile_pool(name="hpool", bufs=4))
        t2pool = ctx.enter_context(tc.tile_pool(name="t2pool", bufs=4))
        opool = ctx.enter_context(tc.tile_pool(name="opool", bufs=2))

        # ---- resident constants -----------------------------------------
        w1_sb = singles.tile([97, 3, 128], bf16)
        nc.sync.dma_start(out=w1_sb, in_=w1s.rearrange("d k m -> k d m"))
        w2a_sb = singles.tile([128, 3, 128], bf16)
        nc.sync.dma_start(out=w2a_sb, in_=w2a.rearrange("d k m -> k d m"))
        # dy=+1 taps live at partitions 64:128 to match the moving operand
        w2b_sb = singles.tile([128, 3, 128], bf16)
        nc.sync.dma_start(out=w2b_sb[64:128], in_=w2b.rearrange("d k m -> k d m"))
        b2v_sb = singles.tile([128, 1], f32)
        nc.sync.dma_start(out=b2v_sb, in_=b2v)
        b2t_sb = singles.tile([128, 1], f32)
        nc.sync.dma_start(out=b2t_sb[64:128], in_=b2t)
        hinit = singles.tile([128, 17, 32], bf16)
        nc.vector.memset(hinit[64:128], G_ZERO)
        h2init = singles.tile([128, 16, 32], bf16)
        nc.vector.memset(h2init[64:128], G_ZERO)

        h1_prev = hinit[64:128, :, :]    # layer-1 state (parts 64:128)
        h2_prev = h2init[64:128, :, :]   # layer-2 state (parts 64:128)
        state = {}
        fr = {}                          # per-frame context dicts

        # Software pipeline, skewed so every op's inputs are at least one
        # emission-iteration old (in-order engine queues never block):
        #   iter i: conv1(i)+sig(i)+zhop(i) | scan1(i-1)+t2a fills(i-1) |
        #           conv2(i-2)+sig2(i-2)+z2hop(i-2) | scan2(i-2, late DVE)
        def st_conv1(t):
            c = fr[t] = {}
            xt = xpool.tile([97, 17, 34], bf16, name="xt", tag="xt")
            nc.sync.dma_start(out=xt, in_=xp[t])
            p1 = c["p1"] = p1_pool.tile([128, 17, 32], f32, name="p1")
            for dx in range(3):
                nc.tensor.matmul(
                    p1[:, 0:16, :], w1_sb[:, dx, :], xt[:, 0:16, dx:dx + 32],
                    start=(dx == 0), stop=(dx == 2))
            for dx in range(3):
                nc.tensor.matmul(
                    p1[:, 16:17, :], w1_sb[:, dx, :], xt[:, 16:17, dx:dx + 32],
                    start=(dx == 0), stop=(dx == 2))

        def st_sig1(t):
            c = fr[t]
            sig = c["sig"] = spool.tile([128, 17, 32], bf16, name="sig", tag="sig")
            nc.scalar.activation(sig, c["p1"], SIG)
            z64 = c["z64"] = spool.tile([128, 17, 32], bf16, name="z64", tag="z64")
            nc.sync.dma_start(out=z64[64:128], in_=sig[0:64])

        def st_scan1(t):
            nonlocal h1_prev
            c = fr[t]
            # g = max(sigmoid(hid), hid + 0.5), fused PSUM read
            g = gpool.tile([128, 17, 32], bf16, tag="g")
            nc.vector.scalar_tensor_tensor(
                g[64:128], c["p1"][64:128], 0.5, c["sig"][64:128], ADD, MAX)
            d = dpool.tile([128, 17, 32], bf16, tag="d")
            nc.vector.tensor_sub(d[64:128], g[64:128], h1_prev)
            m = mpool.tile([128, 17, 32], bf16, tag="m")
            nc.vector.tensor_mul(m[64:128], c["z64"][64:128], d[64:128])
            hst = c["hst"] = hpool.tile([128, 17, 32], bf16, name="hst", tag="hst")
            nc.vector.tensor_add(hst[64:128], h1_prev, m[64:128])
            h1_prev = hst[64:128, :, :]

        def st_fill(t):
            c = fr[t]
            hst = c["hst"]
            # conv2 moving tile: partitions 64:128 = dy=0 group (h1 row r
            # at pos r, dy=+1 read shifted), 0:64 = dy=-1 group (row r at
            # pos r+1, rows 0..14; pos 0 zero)
            t2a = c["t2a"] = t2pool.tile([128, 18, 34], bf16, name="t2a", tag="t2a")
            nc.vector.memset(t2a[:, :, 0:34:33], 0.0)
            nc.vector.memset(t2a[0:64, 0, 1:33], 0.0)
            nc.scalar.activation(
                t2a[64:128, 0:17, 1:33], hst[64:128], COPY, bias=0.0)
            nc.gpsimd.dma_start(
                out=t2a[0:64, 1:16, 1:33], in_=hst[64:128, 0:15, :])

        def st_conv2(t):
            c = fr[t]
            t2a = c["t2a"]
            p2 = c["p2"] = p2_pool.tile([128, 16, 32], f32, name="p2")
            for dx in range(3):
                nc.tensor.matmul(
                    p2, w2a_sb[:, dx, :], t2a[:, 0:16, dx:dx + 32],
                    start=(dx == 0), stop=False)
                nc.tensor.matmul(
                    p2, w2b_sb[64:128, dx, :], t2a[64:128, 1:17, dx:dx + 32],
                    start=False, stop=(dx == 2))

        def st_sig2(t):
            c = fr[t]
            sig2 = c["sig2"] = spool.tile([128, 16, 32], bf16, name="sig2", tag="sig2")
            nc.scalar.activation(sig2, c["p2"], SIG, bias=b2v_sb)
            z2t = c["z2t"] = spool.tile([128, 16, 32], bf16, name="z2t", tag="z2t")
            nc.gpsimd.dma_start(out=z2t[64:128], in_=sig2[0:64])

        def st_scan2(t):
            nonlocal h2_prev
            c = fr[t]
            g2 = gpool.tile([128, 16, 32], bf16, tag="g2")
            nc.vector.scalar_tensor_tensor(
                g2[64:128], c["p2"][64:128], b2t_sb[64:128],
                c["sig2"][64:128], ADD, MAX)
            d2 = dpool.tile([128, 16, 32], bf16, tag="d2")
            nc.vector.tensor_sub(d2[64:128], g2[64:128], h2_prev)
            m2 = mpool.tile([128, 16, 32], bf16, tag="m2")
            nc.vector.tensor_mul(m2[64:128], c["z2t"][64:128], d2[64:128])
            f = t % OB
            if f == 0:
                state["oblk"] = opool.tile(
                    [128, OB, 16, 32], bf16, name="oblk", tag="oblk")
            o2 = state["oblk"]
            nc.vector.tensor_add(o2[64:128, f], h2_prev, m2[64:128])
            if f == OB - 1:
                nc.scalar.dma_start(
                    out=out2o[t - OB + 1:t + 1].rearrange("f c y x -> c f y x"),
                    in_=o2[64:128])
            h2_prev = o2[64:128, f, :, :]
            del fr[t]

        for i in range(S + 3):
            # fills(i-2) first: their input hst(i-2) is a full iteration
            # old, so the ACT/gpsimd queues never block on them
            if 0 <= i - 2 < S:
                st_fill(i - 2)
            if i < S:
                st_conv1(i)
            if 0 <= i - 2 < S:
                st_conv2(i - 2)
            if i < S:
                st_sig1(i)
            if 0 <= i - 2 < S:
                st_sig2(i - 2)
            if 0 <= i - 1 < S:
                st_scan1(i - 1)
            if 0 <= i - 3 < S:
                st_scan2(i - 3)

        nc.sync.dma_start(out=h1o, in_=h1_prev)


def _prep_core_inputs(x, w1, b1, w2, b2, core):
    """Build the per-core input map (numpy, top-core semantics)."""
    bf16 = _np_bf16()
    b = core % 4
    half = core // 4
    if half == 0:
        xl = x[b][:, :, 0:18, :]                      # (S, C, 18, W)
        w1e, w2e = w1, w2
    else:
        xl = x[b][:, :, :13:-1, :]                    # rows 31..14 flipped
        w1e = w1[:, :, ::-1, :]
        w2e = w2[:, :, ::-1, :]

    # dy-im2col with zero pads + ones-row (bias):
    #   xp[t, dyi*32+c, y, 1+xx] = xl[t, c, y-1+dyi, xx];  xp[t, 96] = 1
    xpad = np.zeros((S, C, 19, W + 2), np.float32)
    xpad[:, :, 1:19, 1:33] = xl
    xpc = np.empty((S, 97, 17, 34), np.float32)
    for dyi in range(3):
        xpc[:, dyi * 32:(dyi + 1) * 32] = xpad[:, :, dyi:dyi + 17, :]
    xpc[:, 96] = 1.0

    # conv1 lhsT: [dx, (dyi, c), o]; bias row only in dx=1
    w1t = np.zeros((3, 97, 128), np.float32)
    w1t[:, 0:96] = w1e.transpose(3, 2, 1, 0).reshape(3, 96, 128)
    w1t[1, 96] = b1

    # conv2 lhsT: [dx, (dyi in {0,1}, c), o] packed K=128; dy=+1 separate
    w2t = w2e.transpose(3, 2, 1, 0)                   # (dx, dy, c, o)
    w2at = np.ascontiguousarray(w2t[:, 0:2].reshape(3, 128, 128))
    w2bt = np.ascontiguousarray(w2t[:, 2])

    return {
        "xp": np.ascontiguousarray(xpc).astype(bf16),
        "w1s": w1t.astype(bf16),
        "w2a": w2at.astype(bf16),
        "w2b": w2bt.astype(bf16),
        "b2v": b2.reshape(128, 1).astype(np.float32),
        "b2t": (b2[64:128] + 0.5).reshape(64, 1).astype(np.float32),
    }


def kernel(x, w1, b1, w2, b2):
    global _NC_CACHE, LAST_RESULTS
    from concourse.bass_utils import run_bass_kernel_spmd

    x = np.asarray(x, dtype=np.float32)
    w1 = np.asarray(w1, dtype=np.float32)
    b1 = np.asarray(b1, dtype=np.float32)
    w2 = np.asarray(w2, dtype=np.float32)
    b2 = np.asarray(b2, dtype=np.float32)

    if _NC_CACHE is None:
        _NC_CACHE = _build_nc()
    nc = _NC_CACHE

    in_maps = [_prep_core_inputs(x, w1, b1, w2, b2, i) for i in range(NCORES)]
    res = run_bass_kernel_spmd(nc, in_maps, list(range(NCORES)), trace=TRACE)
    LAST_RESULTS = res

    out2 = np.empty((B, S, HC, H, W), np.float32)
    h1f = np.empty((B, 1, HC, H, W), np.float32)
    for core in range(NCORES):
        b = core % 4
        half = core // 4
        o = np.asarray(res.results[core]["out2o"]).astype(np.float32)
        h1s = np.asarray(res.results[core]["h1o"]).astype(
            np.float32).reshape(64, 17, 32)
        if half == 0:
            out2[b, :, :, 0:16] = o
            h1f[b, 0, :, 0:16] = h1s[:, 0:16]
        else:
            out2[b, :, :, 16:32] = o[:, :, ::-1, :]
            h1f[b, 0, :, 16:32] = h1s[:, 15::-1]
    h2 = out2[:, S - 1:S].copy()
    return out2, h1f, h2
